# revision 5
# baseline (speedup 1.0000x reference)
"""Trainium2 Bass kernel for nn_Decoder (capsule top-1 masking + 3-layer MLP).

Reference computation (per sample b):
    s[b, j]  = sum_u x[b, j, u]^2            (squared capsule norms, j in 0..9)
    jmax     = argmax_j s[b, j]
    v[b]     = flatten(x[b] * onehot(jmax))  # [160], only 16 nonzero
    h1 = relu(v @ W1 + b1)                   # [512]
    h2 = relu(h1 @ W2 + b2)                  # [1024]
    y  = sigmoid(h2 @ W3 + b3)               # [3072]

Distribution: data-parallel over batch across 8 NeuronCores (4096 rows each),
weights replicated. No cross-core communication.

Per-core dataflow (feature-major activations, batch tile of 512):
  All three GEMMs run as fp8(e4m3) DoubleRow matmuls: both operands fp8, two
  128-deep K-subtiles contracted per pass (PE peak 2x bf16/fp32r). Weights are
  pre-scaled by powers of two (s1=4, s2=8, s3=8) when cast to fp8 so their
  small uniform ranges stay out of the e4m3 subnormal region; activations then
  carry the accumulated scale (h1*4, h2*32) and the final sigmoid divides it
  back out via the ACT scale knob (1/256). Validated end to end against the
  fp32 reference: rel-err ~8e-3 vs the 2e-2 gate.

  x tile [128,4,160] -> mask on DVE (5 whole-tile ops) -> fp8 masked x
  -> PE transposes (fp8 identity, 1 cyc/row) -> xT [80,2,512]
  -> L1/L2 DoubleRow matmuls, bias+relu as a single dual-op DVE tensor_scalar
     (max(ps+b, 0)) writing fp8 activations
  -> L3 swapped (h2T stationary / W3 moving) with b3 pre-added by a K=1
     ones-row fp8 matmul into each PSUM group, so the epilogue is a single
     ACT sigmoid (scale=1/256) per [128,1024] PSUM pair, writing fp16.
  y is stored fp16 (halves the 50MB/core output traffic; sigmoid outputs are
  O(1) so fp16 adds ~5e-4 abs err) and upcast to fp32 on the host.

Engine budget per 512-row tile (cost model): PE ~15us (matmul stream),
ACT ~12.6us (12 sigmoids), DVE ~11us (mask+relus), DMA ~15us. Stages are
interleaved across tiles (L1/L2 of tile t+1 emitted between L3 bsub groups of
tile t) so DVE relus hide under the L3 matmul stream.
"""

import os
import sys

import numpy as np

sys.path.insert(0, "/opt/trn_rl_repo")

# Constants (hardcoded per problem spec)
B = 32768
N_CORES = 8
B_SH = B // N_CORES  # 4096 rows per core
TILE_B = 512
D_IN = 160
H1 = 512
H2 = 1024
D_OUT = 3072
N_CAPS = 10
UNIT = 16

# fp8 weight pre-scales (powers of two; see module docstring)
S1 = 4.0
S2 = 8.0
S3 = 8.0
SZ = S1 * S2 * S3  # 256: scale carried by the L3 pre-sigmoid accumulator

_CACHE = {}


def _build_nc(b_sh=B_SH, repeat=1, interleave=True, l3_bias="pe", out_dt="f16"):
    import concourse.bass as bass
    import concourse.mybir as mybir
    import concourse.tile as tile
    from concourse import bacc
    from concourse.masks import make_identity

    n_tiles = b_sh // TILE_B
    dt = mybir.dt
    f32 = dt.float32
    fp8 = dt.float8e4
    ydt = {"f16": dt.float16, "f32": f32}[out_dt]
    AF = mybir.ActivationFunctionType
    AX = mybir.AxisListType
    OP = mybir.AluOpType
    DR = mybir.MatmulPerfMode.DoubleRow

    nc = bacc.Bacc(None, target_bir_lowering=False, debug=False)

    x = nc.dram_tensor("x", [b_sh, D_IN], f32, kind="ExternalInput").ap()
    W1 = nc.dram_tensor("W1", [D_IN, H1], f32, kind="ExternalInput").ap()
    b1 = nc.dram_tensor("b1", [H1], f32, kind="ExternalInput").ap()
    W2 = nc.dram_tensor("W2", [H1, H2], f32, kind="ExternalInput").ap()
    b2 = nc.dram_tensor("b2", [H2], f32, kind="ExternalInput").ap()
    W3 = nc.dram_tensor("W3", [H2, D_OUT], f32, kind="ExternalInput").ap()
    b3 = nc.dram_tensor("b3", [D_OUT], f32, kind="ExternalInput").ap()
    y = nc.dram_tensor("y", [b_sh, D_OUT], ydt, kind="ExternalOutput").ap()

    with tile.TileContext(nc) as tc:
        with (
            tc.tile_pool(name="singles", bufs=1) as singles,
            tc.tile_pool(name="xin", bufs=2) as xin,
            tc.tile_pool(name="mtmp", bufs=2) as mtmp,
            tc.tile_pool(name="xtp", bufs=2) as xtp,
            tc.tile_pool(name="acts", bufs=2) as acts,
            tc.tile_pool(name="yout", bufs=2) as yout,
            tc.tile_pool(name="psum_mm", bufs=2, space="PSUM") as pp,
            tc.tile_pool(name="psum_l3", bufs=2, space="PSUM") as pl3,
            tc.tile_pool(name="psum_tr", bufs=2, space="PSUM") as ptr,
        ):
            # ---- one-time setup: identity, biases, weights ----
            ident16 = singles.tile([128, 128], dt.bfloat16)
            make_identity(nc, ident16)

            # Small bias loads first: they gate tile-0's L1/L2 relu and must
            # not queue behind 15MB of weights on the ACT HWDGE queue.
            b1s = singles.tile([128, 4], f32)  # b1s[p, m] = S1 * b1[m*128+p]
            nc.scalar.dma_start(out=b1s, in_=b1.rearrange("(m p) -> p m", p=128))
            nc.vector.tensor_scalar_mul(b1s, b1s, S1)
            b2s = singles.tile([128, 8], f32)
            nc.scalar.dma_start(out=b2s, in_=b2.rearrange("(m p) -> p m", p=128))
            nc.vector.tensor_scalar_mul(b2s, b2s, S1 * S2)

            # L3 bias delivered through the PE: a K=1 DoubleRow matmul of
            # ones[1,128] x (SZ*b3)[1,n-slice] accumulated first into each
            # PSUM group (sub-tile 1 of the pair is zeros).
            ones8 = singles.tile([1, 2, 128], fp8)
            nc.gpsimd.memset(ones8[:, 0, :], 1.0)
            nc.gpsimd.memset(ones8[:, 1, :], 0.0)
            b3x = singles.tile([1, 2, D_OUT], fp8)
            nc.gpsimd.memset(b3x[:, 1, :], 0.0)

            w1 = singles.tile([80, 2, H1], fp8)  # sub k: W1[80k:80(k+1), :]
            w2 = singles.tile([128, 4, H2], fp8)  # [p, ko, n]
            w3 = singles.tile([128, 8, D_OUT], fp8)

            # Weight fp8 casts ride GpSimd (1-input streaming at line rate,
            # keeps DVE free for the mask/relu pipeline); DMAs ride the ACT
            # HWDGE queue so tile-0's x DMA on the SP queue isn't stuck
            # behind 15MB of weights.
            with tc.tile_pool(name="wstage", bufs=2) as wstage:

                def load_cast(dst, src, scale):
                    p, fsz = src.shape[0], int(np.prod(src.shape[1:]))
                    st = wstage.tile([128, 1536], f32)
                    nc.scalar.dma_start(out=st[:p, :fsz], in_=src)
                    nc.gpsimd.tensor_scalar_mul(dst, st[:p, :fsz], scale)

                load_cast(w1[:, 0, :], W1[0:80, :], S1)
                load_cast(w1[:, 1, :], W1[80:160, :], S1)
                for k in range(4):
                    load_cast(w2[:, k, :], W2[k * 128 : (k + 1) * 128, :], S2)
                stb = wstage.tile([1, D_OUT], f32)
                nc.scalar.dma_start(
                    out=stb, in_=b3.rearrange("(p d) -> p d", p=1)
                )
                nc.gpsimd.tensor_scalar_mul(b3x[:, 0, :], stb, SZ)
                # W3 in n2-order (1024-col blocks) so L3's first n-group
                # unblocks as early as possible.
                for n2 in range(3):
                    cs = slice(n2 * 1024, (n2 + 1) * 1024)
                    for k in range(8):
                        load_cast(w3[:, k, cs], W3[k * 128 : (k + 1) * 128, cs], S3)

            def front(t):
                """x load -> mask -> fp8 -> PE transposes -> xT for tile t."""
                r0 = (t % n_tiles) * TILE_B
                # x tile: [128, 4, 160], sub s holds rows r0+s*128 ...
                x_t = xin.tile([128, 4, D_IN], f32)
                nc.sync.dma_start(
                    out=x_t,
                    in_=x[r0 : r0 + TILE_B, :].rearrange("(s p) d -> p s d", p=128),
                )
                # whole-tile mask pipeline on DVE (5 ops)
                sq = mtmp.tile([128, 4, D_IN], f32)
                nc.vector.tensor_tensor(sq, x_t, x_t, op=OP.mult)
                s10 = mtmp.tile([128, 4, N_CAPS], f32)
                nc.vector.reduce_sum(
                    s10, sq.rearrange("p s (g u) -> p s g u", u=UNIT), axis=AX.X
                )
                mx = mtmp.tile([128, 4], f32)
                nc.vector.reduce_max(mx, s10, axis=AX.X)
                msk = mtmp.tile([128, 4, N_CAPS], f32)
                nc.vector.tensor_tensor(
                    msk, s10, mx.broadcast_to([128, 4, N_CAPS]), op=OP.is_ge
                )
                xm = mtmp.tile([128, 4, D_IN], dt.bfloat16)
                nc.vector.tensor_tensor(
                    xm.rearrange("p s (g u) -> p s g u", u=UNIT),
                    x_t.rearrange("p s (g u) -> p s g u", u=UNIT),
                    msk.broadcast_to([128, 4, N_CAPS, UNIT]),
                    op=OP.mult,
                )
                # transpose to feature-major [80, 2, TILE_B] (bf16, 1 cyc/row;
                # fp8 transpose needs output element step 2, so cast to fp8 in
                # the PSUM->SBUF copy instead)
                tp = ptr.tile([80, 2, TILE_B], dt.bfloat16)
                for s in range(4):
                    bs = slice(s * 128, (s + 1) * 128)
                    nc.tensor.transpose(tp[:, 0, bs], xm[:, s, 0:80], ident16)
                    nc.tensor.transpose(tp[:, 1, bs], xm[:, s, 80:160], ident16)
                xT = xtp.tile([80, 2, TILE_B], fp8)
                nc.vector.tensor_copy(xT, tp)
                return xT

            def l1(xT):
                """L1: single DoubleRow matmul per m chunk (K=2x80), then
                bias+relu as one dual-op DVE tensor_scalar -> fp8 h1T."""
                h1T = acts.tile([128, 4, TILE_B], fp8)
                for m in range(4):
                    ps = pp.tile([128, TILE_B], f32)
                    nc.tensor.matmul(
                        ps,
                        w1[:, :, m * 128 : (m + 1) * 128],
                        xT,
                        start=True,
                        stop=True,
                        perf_mode=DR,
                    )
                    nc.vector.tensor_scalar(
                        h1T[:, m, :], ps, b1s[:, m : m + 1], 0.0,
                        op0=OP.add, op1=OP.max,
                    )
                return h1T

            def l2(h1T, h2T, ms):
                """L2 m-chunks in ms: 2 DoubleRow matmuls (K=4x128), then
                bias+relu on DVE -> fp8 h2T."""
                for m in ms:
                    ps = pp.tile([128, TILE_B], f32)
                    for kp in range(2):
                        nc.tensor.matmul(
                            ps,
                            w2[:, 2 * kp : 2 * kp + 2, m * 128 : (m + 1) * 128],
                            h1T[:, 2 * kp : 2 * kp + 2, :],
                            start=(kp == 0),
                            stop=(kp == 1),
                            perf_mode=DR,
                        )
                    nc.vector.tensor_scalar(
                        h2T[:, m, :], ps, b2s[:, m : m + 1], 0.0,
                        op0=OP.add, op1=OP.max,
                    )

            def l3_bsub(h2T, t, bsub, y_t):
                """L3 for one 128-row batch sub-block: 3 PSUM pairs of
                [128,1024]; each 512-wide group = bias matmul + 4 DoubleRow
                matmuls; epilogue = one ACT sigmoid per pair -> fp16 y_t."""
                r0 = (t % n_tiles) * TILE_B
                hs = h2T[:, :, bsub * 128 : (bsub + 1) * 128]
                for n2 in range(3):
                    ps = pl3.tile([128, 2, TILE_B], f32)
                    for h in range(2):
                        nsl = slice((2 * n2 + h) * TILE_B, (2 * n2 + h + 1) * TILE_B)
                        if l3_bias == "pe":
                            nc.tensor.matmul(
                                ps[:, h, :], ones8, b3x[:, :, nsl],
                                start=True, stop=False, perf_mode=DR,
                            )
                        for kp in range(4):
                            nc.tensor.matmul(
                                ps[:, h, :],
                                hs[:, 2 * kp : 2 * kp + 2, :],
                                w3[:, 2 * kp : 2 * kp + 2, nsl],
                                start=(kp == 0 and l3_bias != "pe"),
                                stop=(kp == 3),
                                perf_mode=DR,
                            )
                    ysl = y_t[:, 2 * n2 * TILE_B : 2 * (n2 + 1) * TILE_B]
                    if l3_bias == "pe":
                        nc.scalar.activation(ysl, ps, AF.Sigmoid, scale=1.0 / SZ)
                    else:
                        nc.vector.tensor_scalar(
                            ysl, ps, 1.0 / SZ, None, op0=OP.mult
                        )
                        nc.scalar.activation(ysl, ysl, AF.Sigmoid)
                nc.sync.dma_start(
                    out=y[r0 + bsub * 128 : r0 + (bsub + 1) * 128, :], in_=y_t
                )

            total_tiles = n_tiles * repeat

            if not interleave:
                for t in range(total_tiles):
                    xT = front(t)
                    h1T = l1(xT)
                    h2T = acts.tile([128, 8, TILE_B], fp8)
                    l2(h1T, h2T, range(8))
                    for bsub in range(4):
                        y_t = yout.tile([128, D_OUT], ydt)
                        l3_bsub(h2T, t, bsub, y_t)
            else:
                # software pipeline: L1/L2 of tile t+1 interleave with the L3
                # matmul stream of tile t so the DVE relus hide under PE work.
                xT = front(0)
                h1T = l1(xT)
                h2T = acts.tile([128, 8, TILE_B], fp8)
                l2(h1T, h2T, range(8))
                for t in range(total_tiles):
                    last = t + 1 >= total_tiles
                    if not last:
                        xT = front(t + 1)
                    y_ts = [
                        yout.tile([128, D_OUT], ydt, name=f"y_t{i}", tag="y_t")
                        for i in range(4)
                    ]
                    l3_bsub(h2T, t, 0, y_ts[0])
                    l3_bsub(h2T, t, 1, y_ts[1])
                    if not last:
                        h1T_n = l1(xT)
                    l3_bsub(h2T, t, 2, y_ts[2])
                    if not last:
                        h2T_n = acts.tile([128, 8, TILE_B], fp8)
                        l2(h1T_n, h2T_n, range(0, 4))
                    l3_bsub(h2T, t, 3, y_ts[3])
                    if not last:
                        l2(h1T_n, h2T_n, range(4, 8))
                        h2T = h2T_n

    nc.finalize()
    return nc


def _get_nc():
    key = (
        os.environ.get("DEC_INTERLEAVE", "1"),
        os.environ.get("DEC_L3_BIAS", "pe"),
        os.environ.get("DEC_OUT_DTYPE", "f16"),
    )
    if key not in _CACHE:
        _CACHE[key] = _build_nc(
            interleave=key[0] == "1", l3_bias=key[1], out_dt=key[2]
        )
    return _CACHE[key]


def kernel(**inputs):
    from concourse.bass_utils import run_bass_kernel_spmd

    x = np.ascontiguousarray(np.asarray(inputs["x"], dtype=np.float32)).reshape(
        B, D_IN
    )
    W1 = np.asarray(inputs["W1"], dtype=np.float32)
    b1 = np.asarray(inputs["b1"], dtype=np.float32)
    W2 = np.asarray(inputs["W2"], dtype=np.float32)
    b2 = np.asarray(inputs["b2"], dtype=np.float32)
    W3 = np.asarray(inputs["W3"], dtype=np.float32)
    b3 = np.asarray(inputs["b3"], dtype=np.float32)

    nc = _get_nc()

    in_maps = []
    for c in range(N_CORES):
        in_maps.append(
            {
                "x": x[c * B_SH : (c + 1) * B_SH],
                "W1": W1,
                "b1": b1,
                "W2": W2,
                "b2": b2,
                "W3": W3,
                "b3": b3,
            }
        )
    res = run_bass_kernel_spmd(
        nc,
        in_maps,
        list(range(N_CORES)),
        trace=bool(int(os.environ.get("DEC_TRACE", "0"))),
    )
    out = np.concatenate(
        [np.asarray(res.results[c]["y"]) for c in range(N_CORES)], axis=0
    ).astype(np.float32)
    kernel.last_exec_time_ns = res.exec_time_ns
    kernel.last_results = res
    return out


# revision 6
# speedup vs baseline: 2.1224x; 2.1224x over previous
"""Trainium2 Bass kernel for nn_Decoder (capsule top-1 masking + 3-layer MLP).

Reference computation (per sample b):
    s[b, j]  = sum_u x[b, j, u]^2            (squared capsule norms, j in 0..9)
    jmax     = argmax_j s[b, j]
    v[b]     = flatten(x[b] * onehot(jmax))  # [160], only 16 nonzero
    h1 = relu(v @ W1 + b1)                   # [512]
    h2 = relu(h1 @ W2 + b2)                  # [1024]
    y  = sigmoid(h2 @ W3 + b3)               # [3072]

Distribution: data-parallel over batch across 8 NeuronCores (4096 rows each),
weights replicated. No cross-core communication.

Per-core dataflow (feature-major activations, batch tile of 512):
  All three GEMMs run as fp8(e4m3) DoubleRow matmuls: both operands fp8, two
  128-deep K-subtiles contracted per pass (PE peak 2x bf16/fp32r). Weights are
  pre-scaled by powers of two (s1=4, s2=8, s3=8) when cast to fp8 so their
  small uniform ranges stay out of the e4m3 subnormal region; activations then
  carry the accumulated scale (h1*4, h2*32) and the final sigmoid divides it
  back out via the ACT scale knob (1/256). Validated end to end against the
  fp32 reference: rel-err ~8e-3 vs the 2e-2 gate.

  x tile [128,4,160] -> mask on DVE (5 whole-tile ops) -> bf16 masked x
  -> PE transposes (bf16, 1 cyc/row; fp8-cast in the PSUM->SBUF copy)
  -> xT [80,2,512] fp8
  -> L1/L2 DoubleRow matmuls, bias+relu as a single dual-op DVE tensor_scalar
     (max(ps+b, 0)) writing fp8 activations
  -> L3 swapped (h2T stationary / W3 moving) with b3 pre-added by a K=1
     ones-row fp8 matmul into each PSUM group, so the epilogue is a single
     ACT sigmoid (scale=1/256) per [128,1024] PSUM pair, writing fp16.
  y is stored fp16 (halves the 50MB/core output traffic; sigmoid outputs are
  O(1) so fp16 adds ~5e-4 abs err) and upcast to fp32 on the host.

Engine budget per 512-row tile (cost model): PE ~15us (matmul stream),
ACT ~12.6us (12 sigmoids), DVE ~11us (mask+relus), DMA ~15us. Stages are
interleaved across tiles (L1/L2 of tile t+1 emitted between L3 bsub groups of
tile t) so DVE relus hide under the L3 matmul stream.

Measured (repetition-slope on hardware, bench_slope.py): ~245us/core
(min-based; median-of-pairs 152us) vs 520us for the previous f32r kernel.
TimelineSim cost model predicts 221us (PE 134us busy, DMA 120us, ACT 102us,
DVE 92us). End-to-end rel-err on hardware: 7.7e-3 (gate 2e-2).
"""

import os
import sys

import numpy as np

sys.path.insert(0, "/opt/trn_rl_repo")

# Constants (hardcoded per problem spec)
B = 32768
N_CORES = 8
B_SH = B // N_CORES  # 4096 rows per core
TILE_B = 512
D_IN = 160
H1 = 512
H2 = 1024
D_OUT = 3072
N_CAPS = 10
UNIT = 16

# fp8 weight pre-scales (powers of two; see module docstring)
S1 = 4.0
S2 = 8.0
S3 = 8.0
SZ = S1 * S2 * S3  # 256: scale carried by the L3 pre-sigmoid accumulator

_CACHE = {}


def _build_nc(b_sh=B_SH, repeat=1, interleave=True, l3_bias="pe", out_dt="f16"):
    import concourse.bass as bass
    import concourse.mybir as mybir
    import concourse.tile as tile
    from concourse import bacc
    from concourse.masks import make_identity

    n_tiles = b_sh // TILE_B
    dt = mybir.dt
    f32 = dt.float32
    fp8 = dt.float8e4
    ydt = {"f16": dt.float16, "f32": f32}[out_dt]
    AF = mybir.ActivationFunctionType
    AX = mybir.AxisListType
    OP = mybir.AluOpType
    DR = mybir.MatmulPerfMode.DoubleRow

    nc = bacc.Bacc(None, target_bir_lowering=False, debug=False)

    x = nc.dram_tensor("x", [b_sh, D_IN], f32, kind="ExternalInput").ap()
    W1 = nc.dram_tensor("W1", [D_IN, H1], f32, kind="ExternalInput").ap()
    b1 = nc.dram_tensor("b1", [H1], f32, kind="ExternalInput").ap()
    W2 = nc.dram_tensor("W2", [H1, H2], f32, kind="ExternalInput").ap()
    b2 = nc.dram_tensor("b2", [H2], f32, kind="ExternalInput").ap()
    W3 = nc.dram_tensor("W3", [H2, D_OUT], f32, kind="ExternalInput").ap()
    b3 = nc.dram_tensor("b3", [D_OUT], f32, kind="ExternalInput").ap()
    y = nc.dram_tensor("y", [b_sh, D_OUT], ydt, kind="ExternalOutput").ap()

    with tile.TileContext(nc) as tc:
        with (
            tc.tile_pool(name="singles", bufs=1) as singles,
            tc.tile_pool(name="xin", bufs=2) as xin,
            tc.tile_pool(name="mtmp", bufs=2) as mtmp,
            tc.tile_pool(name="xtp", bufs=2) as xtp,
            tc.tile_pool(name="acts", bufs=2) as acts,
            tc.tile_pool(name="yout", bufs=2) as yout,
            tc.tile_pool(name="psum_mm", bufs=2, space="PSUM") as pp,
            tc.tile_pool(name="psum_l3", bufs=2, space="PSUM") as pl3,
            tc.tile_pool(name="psum_tr", bufs=2, space="PSUM") as ptr,
        ):
            # ---- one-time setup: identity, biases, weights ----
            ident16 = singles.tile([128, 128], dt.bfloat16)
            make_identity(nc, ident16)

            # Small bias loads first: they gate tile-0's L1/L2 relu and must
            # not queue behind 15MB of weights on the ACT HWDGE queue.
            b1s = singles.tile([128, 4], f32)  # b1s[p, m] = S1 * b1[m*128+p]
            nc.scalar.dma_start(out=b1s, in_=b1.rearrange("(m p) -> p m", p=128))
            nc.vector.tensor_scalar_mul(b1s, b1s, S1)
            b2s = singles.tile([128, 8], f32)
            nc.scalar.dma_start(out=b2s, in_=b2.rearrange("(m p) -> p m", p=128))
            nc.vector.tensor_scalar_mul(b2s, b2s, S1 * S2)

            # L3 bias delivered through the PE: a K=1 DoubleRow matmul of
            # ones[1,128] x (SZ*b3)[1,n-slice] accumulated first into each
            # PSUM group (sub-tile 1 of the pair is zeros).
            ones8 = singles.tile([1, 2, 128], fp8)
            nc.gpsimd.memset(ones8[:, 0, :], 1.0)
            nc.gpsimd.memset(ones8[:, 1, :], 0.0)
            b3x = singles.tile([1, 2, D_OUT], fp8)
            nc.gpsimd.memset(b3x[:, 1, :], 0.0)

            w1 = singles.tile([80, 2, H1], fp8)  # sub k: W1[80k:80(k+1), :]
            w2 = singles.tile([128, 4, H2], fp8)  # [p, ko, n]
            w3 = singles.tile([128, 8, D_OUT], fp8)

            # Weight fp8 casts ride GpSimd (1-input streaming at line rate,
            # keeps DVE free for the mask/relu pipeline); DMAs ride the ACT
            # HWDGE queue so tile-0's x DMA on the SP queue isn't stuck
            # behind 15MB of weights.
            with tc.tile_pool(name="wstage", bufs=2) as wstage:

                def load_cast(dst, src, scale):
                    p, fsz = src.shape[0], int(np.prod(src.shape[1:]))
                    st = wstage.tile([128, 1536], f32)
                    nc.scalar.dma_start(out=st[:p, :fsz], in_=src)
                    nc.gpsimd.tensor_scalar_mul(dst, st[:p, :fsz], scale)

                load_cast(w1[:, 0, :], W1[0:80, :], S1)
                load_cast(w1[:, 1, :], W1[80:160, :], S1)
                for k in range(4):
                    load_cast(w2[:, k, :], W2[k * 128 : (k + 1) * 128, :], S2)
                stb = wstage.tile([1, D_OUT], f32)
                nc.scalar.dma_start(
                    out=stb, in_=b3.rearrange("(p d) -> p d", p=1)
                )
                nc.gpsimd.tensor_scalar_mul(b3x[:, 0, :], stb, SZ)
                # W3 in n2-order (1024-col blocks) so L3's first n-group
                # unblocks as early as possible.
                for n2 in range(3):
                    cs = slice(n2 * 1024, (n2 + 1) * 1024)
                    for k in range(8):
                        load_cast(w3[:, k, cs], W3[k * 128 : (k + 1) * 128, cs], S3)

            def front(t):
                """x load -> mask -> fp8 -> PE transposes -> xT for tile t."""
                r0 = (t % n_tiles) * TILE_B
                # x tile: [128, 4, 160], sub s holds rows r0+s*128 ...
                x_t = xin.tile([128, 4, D_IN], f32)
                nc.sync.dma_start(
                    out=x_t,
                    in_=x[r0 : r0 + TILE_B, :].rearrange("(s p) d -> p s d", p=128),
                )
                # whole-tile mask pipeline on DVE (5 ops)
                sq = mtmp.tile([128, 4, D_IN], f32)
                nc.vector.tensor_tensor(sq, x_t, x_t, op=OP.mult)
                s10 = mtmp.tile([128, 4, N_CAPS], f32)
                nc.vector.reduce_sum(
                    s10, sq.rearrange("p s (g u) -> p s g u", u=UNIT), axis=AX.X
                )
                mx = mtmp.tile([128, 4], f32)
                nc.vector.reduce_max(mx, s10, axis=AX.X)
                msk = mtmp.tile([128, 4, N_CAPS], f32)
                nc.vector.tensor_tensor(
                    msk, s10, mx.broadcast_to([128, 4, N_CAPS]), op=OP.is_ge
                )
                xm = mtmp.tile([128, 4, D_IN], dt.bfloat16)
                nc.vector.tensor_tensor(
                    xm.rearrange("p s (g u) -> p s g u", u=UNIT),
                    x_t.rearrange("p s (g u) -> p s g u", u=UNIT),
                    msk.broadcast_to([128, 4, N_CAPS, UNIT]),
                    op=OP.mult,
                )
                # transpose to feature-major [80, 2, TILE_B] (bf16, 1 cyc/row;
                # fp8 transpose needs output element step 2, so cast to fp8 in
                # the PSUM->SBUF copy instead)
                tp = ptr.tile([80, 2, TILE_B], dt.bfloat16)
                for s in range(4):
                    bs = slice(s * 128, (s + 1) * 128)
                    nc.tensor.transpose(tp[:, 0, bs], xm[:, s, 0:80], ident16)
                    nc.tensor.transpose(tp[:, 1, bs], xm[:, s, 80:160], ident16)
                xT = xtp.tile([80, 2, TILE_B], fp8)
                nc.vector.tensor_copy(xT, tp)
                return xT

            def l1(xT):
                """L1: single DoubleRow matmul per m chunk (K=2x80), then
                bias+relu as one dual-op DVE tensor_scalar -> fp8 h1T."""
                h1T = acts.tile([128, 4, TILE_B], fp8)
                for m in range(4):
                    ps = pp.tile([128, TILE_B], f32)
                    nc.tensor.matmul(
                        ps,
                        w1[:, :, m * 128 : (m + 1) * 128],
                        xT,
                        start=True,
                        stop=True,
                        perf_mode=DR,
                    )
                    nc.vector.tensor_scalar(
                        h1T[:, m, :], ps, b1s[:, m : m + 1], 0.0,
                        op0=OP.add, op1=OP.max,
                    )
                return h1T

            def l2(h1T, h2T, ms):
                """L2 m-chunks in ms: 2 DoubleRow matmuls (K=4x128), then
                bias+relu on DVE -> fp8 h2T."""
                for m in ms:
                    ps = pp.tile([128, TILE_B], f32)
                    for kp in range(2):
                        nc.tensor.matmul(
                            ps,
                            w2[:, 2 * kp : 2 * kp + 2, m * 128 : (m + 1) * 128],
                            h1T[:, 2 * kp : 2 * kp + 2, :],
                            start=(kp == 0),
                            stop=(kp == 1),
                            perf_mode=DR,
                        )
                    nc.vector.tensor_scalar(
                        h2T[:, m, :], ps, b2s[:, m : m + 1], 0.0,
                        op0=OP.add, op1=OP.max,
                    )

            def l3_bsub(h2T, t, bsub, y_t):
                """L3 for one 128-row batch sub-block: 3 PSUM pairs of
                [128,1024]; each 512-wide group = bias matmul + 4 DoubleRow
                matmuls; epilogue = one ACT sigmoid per pair -> fp16 y_t."""
                r0 = (t % n_tiles) * TILE_B
                hs = h2T[:, :, bsub * 128 : (bsub + 1) * 128]
                for n2 in range(3):
                    ps = pl3.tile([128, 2, TILE_B], f32)
                    for h in range(2):
                        nsl = slice((2 * n2 + h) * TILE_B, (2 * n2 + h + 1) * TILE_B)
                        if l3_bias == "pe":
                            nc.tensor.matmul(
                                ps[:, h, :], ones8, b3x[:, :, nsl],
                                start=True, stop=False, perf_mode=DR,
                            )
                        for kp in range(4):
                            nc.tensor.matmul(
                                ps[:, h, :],
                                hs[:, 2 * kp : 2 * kp + 2, :],
                                w3[:, 2 * kp : 2 * kp + 2, nsl],
                                start=(kp == 0 and l3_bias != "pe"),
                                stop=(kp == 3),
                                perf_mode=DR,
                            )
                    ysl = y_t[:, 2 * n2 * TILE_B : 2 * (n2 + 1) * TILE_B]
                    if l3_bias == "pe":
                        nc.scalar.activation(ysl, ps, AF.Sigmoid, scale=1.0 / SZ)
                    else:
                        nc.vector.tensor_scalar(
                            ysl, ps, 1.0 / SZ, None, op0=OP.mult
                        )
                        nc.scalar.activation(ysl, ysl, AF.Sigmoid)
                nc.sync.dma_start(
                    out=y[r0 + bsub * 128 : r0 + (bsub + 1) * 128, :], in_=y_t
                )

            total_tiles = n_tiles * repeat

            if not interleave:
                for t in range(total_tiles):
                    xT = front(t)
                    h1T = l1(xT)
                    h2T = acts.tile([128, 8, TILE_B], fp8)
                    l2(h1T, h2T, range(8))
                    for bsub in range(4):
                        y_t = yout.tile([128, D_OUT], ydt)
                        l3_bsub(h2T, t, bsub, y_t)
            else:
                # software pipeline: L1/L2 of tile t+1 interleave with the L3
                # matmul stream of tile t so the DVE relus hide under PE work.
                xT = front(0)
                h1T = l1(xT)
                h2T = acts.tile([128, 8, TILE_B], fp8)
                l2(h1T, h2T, range(8))
                for t in range(total_tiles):
                    last = t + 1 >= total_tiles
                    if not last:
                        xT = front(t + 1)
                    y_ts = [
                        yout.tile([128, D_OUT], ydt, name=f"y_t{i}", tag="y_t")
                        for i in range(4)
                    ]
                    l3_bsub(h2T, t, 0, y_ts[0])
                    l3_bsub(h2T, t, 1, y_ts[1])
                    if not last:
                        h1T_n = l1(xT)
                    l3_bsub(h2T, t, 2, y_ts[2])
                    if not last:
                        h2T_n = acts.tile([128, 8, TILE_B], fp8)
                        l2(h1T_n, h2T_n, range(0, 4))
                    l3_bsub(h2T, t, 3, y_ts[3])
                    if not last:
                        l2(h1T_n, h2T_n, range(4, 8))
                        h2T = h2T_n

    nc.finalize()
    return nc


def _get_nc():
    key = (
        os.environ.get("DEC_INTERLEAVE", "1"),
        os.environ.get("DEC_L3_BIAS", "pe"),
        os.environ.get("DEC_OUT_DTYPE", "f16"),
    )
    if key not in _CACHE:
        _CACHE[key] = _build_nc(
            interleave=key[0] == "1", l3_bias=key[1], out_dt=key[2]
        )
    return _CACHE[key]


def kernel(**inputs):
    from concourse.bass_utils import run_bass_kernel_spmd

    x = np.ascontiguousarray(np.asarray(inputs["x"], dtype=np.float32)).reshape(
        B, D_IN
    )
    W1 = np.asarray(inputs["W1"], dtype=np.float32)
    b1 = np.asarray(inputs["b1"], dtype=np.float32)
    W2 = np.asarray(inputs["W2"], dtype=np.float32)
    b2 = np.asarray(inputs["b2"], dtype=np.float32)
    W3 = np.asarray(inputs["W3"], dtype=np.float32)
    b3 = np.asarray(inputs["b3"], dtype=np.float32)

    nc = _get_nc()

    in_maps = []
    for c in range(N_CORES):
        in_maps.append(
            {
                "x": x[c * B_SH : (c + 1) * B_SH],
                "W1": W1,
                "b1": b1,
                "W2": W2,
                "b2": b2,
                "W3": W3,
                "b3": b3,
            }
        )
    res = run_bass_kernel_spmd(
        nc,
        in_maps,
        list(range(N_CORES)),
        trace=bool(int(os.environ.get("DEC_TRACE", "0"))),
    )
    out = np.concatenate(
        [np.asarray(res.results[c]["y"]) for c in range(N_CORES)], axis=0
    ).astype(np.float32)
    kernel.last_exec_time_ns = res.exec_time_ns
    kernel.last_results = res
    return out
